# revision 4
# baseline (speedup 1.0000x reference)
"""Cross-attention Trainium2 kernel (8-core SPMD, no collectives).

Problem: tokens [4,4096,320], context [4,4096,768],
  Q = tokens @ WqT, K = ctx @ WkT, V = ctx @ WvT,
  out = softmax(Q K^T / 8) @ V          -> [4,4096,320] f32

Sharding: core c handles batch b=c//2, query rows t in [th*2048,(th+1)*2048),
th=c%2. Each core needs the full context of its batch (K/V duplicated across
the 2 cores of a batch pair); output shards are disjoint -> no collectives.

Device algorithm (per core), all matmuls fp32r (full PE speed at N>=256):
  tokTp [384,2048]  = tokens-slice^T zero-padded (h 320->384)
  ctxT  [768,4096]  = context^T
  QT [128,2048] (e padded 64->128), KT [128,4096]: proj matmuls contract
    over hidden/ctx k-tiles on partitions.
  V  [s,320] per 128-s-tile, stored as Vplus [128, 32, 322] with col 320 = 1.
  Attention per 512-wide t-chunk:
    for each s-tile: scoresT[s,tch] = KT-tile^T(lhsT) @ QT-chunk  (PSUM)
                     expT = exp(0.125*scoresT)   (ACT, PSUM->SBUF)
                     4x AV matmul: av[t128] += expT-slice^T @ Vplus[s-tile]
    av[:,320] accumulates the softmax denominator (ones column trick);
    out = av[:,0:320] * (1/av[:,320]).
No row-max subtraction: |scores| <= ~2 so exp is safely in f32 range.
"""

import numpy as np
from contextlib import ExitStack

import concourse.bass as bass
import concourse.bacc as bacc
import concourse.mybir as mybir
import concourse.tile as tile
from concourse.bass_utils import run_bass_kernel_spmd

P = 128
F32 = mybir.dt.float32
F32R = mybir.dt.float32r

B, T, S_FULL = 4, 4096, 4096
HID, CTX, E = 320, 768, 64
NCORES = 8
TC = T // 2  # 2048 query rows per core


def build_cross_attn(TCc=TC, S=S_FULL, HIDc=HID, CTXc=CTX):
    KH = (HIDc + P - 1) // P       # hidden k-tiles (zero-padded)
    KC = CTXc // P                 # context k-tiles
    TCW = min(512, TCc)            # t-chunk width for scores
    NTCH = TCc // TCW
    T128 = TCW // P                # 128-t subchunks per t-chunk
    ST = S // P                    # s-tiles
    SBLK = min(1024, S)            # context stream block (s columns)
    NSB = S // SBLK
    STB = SBLK // P                # s-tiles per block
    KTW = min(512, SBLK)           # KT chunk width
    NKTC = SBLK // KTW
    QW = min(512, TCc)             # QT chunk width
    HD = HIDc
    HD1 = HD + 2  # +2: ones col at HD, pad col (f32r matmul needs even free dim)

    nc = bacc.Bacc()
    tokT = nc.dram_tensor("tokT", [KH * P, TCc], F32R, kind="ExternalInput")
    ctxT = nc.dram_tensor("ctxT", [CTXc, S], F32R, kind="ExternalInput")
    wqT = nc.dram_tensor("wqT", [KH * P, P], F32R, kind="ExternalInput")
    wkT = nc.dram_tensor("wkT", [CTXc, P], F32R, kind="ExternalInput")
    wvT = nc.dram_tensor("wvT", [CTXc, HD], F32R, kind="ExternalInput")
    out = nc.dram_tensor("out", [TCc, HD], F32, kind="ExternalOutput")

    with ExitStack() as ctx:
        tc = ctx.enter_context(tile.TileContext(nc))
        consts = ctx.enter_context(tc.tile_pool(name="consts", bufs=1))
        ctxp = ctx.enter_context(tc.tile_pool(name="ctxp", bufs=2))
        expp = ctx.enter_context(tc.tile_pool(name="expp", bufs=3))
        outp = ctx.enter_context(tc.tile_pool(name="outp", bufs=4))
        pp = ctx.enter_context(tc.tile_pool(name="pp", bufs=2, space="PSUM"))
        ps = ctx.enter_context(tc.tile_pool(name="ps", bufs=2, space="PSUM"))
        pa = ctx.enter_context(tc.tile_pool(name="pa", bufs=T128, space="PSUM"))

        wq_sb = consts.tile([P, KH, P], F32R)
        nc.sync.dma_start(out=wq_sb, in_=wqT.rearrange("(k p) e -> p k e", p=P))
        wk_sb = consts.tile([P, KC, P], F32R)
        nc.sync.dma_start(out=wk_sb, in_=wkT.rearrange("(k p) e -> p k e", p=P))
        wv_sb = consts.tile([P, KC, HD], F32R)
        nc.sync.dma_start(out=wv_sb, in_=wvT.rearrange("(k p) h -> p k h", p=P))
        tok_sb = consts.tile([P, KH, TCc], F32R)
        nc.sync.dma_start(out=tok_sb, in_=tokT.rearrange("(k p) t -> p k t", p=P))

        qt_sb = consts.tile([P, TCc], F32R)
        kt_sb = consts.tile([P, S], F32R)
        vp_sb = consts.tile([P, ST, HD1], F32R)

        # ---- Q^T = WqT.T @ tokT ----
        for chn in range(TCc // QW):
            qp = pp.tile([P, QW], F32, tag="proj", name="qp")
            for k in range(KH):
                nc.tensor.matmul(
                    qp,
                    lhsT=wq_sb[:, k, :],
                    rhs=tok_sb[:, k, chn * QW:(chn + 1) * QW],
                    start=(k == 0),
                    stop=(k == KH - 1),
                )
            nc.vector.tensor_copy(qt_sb[:, chn * QW:(chn + 1) * QW], qp)

        # softmax-denominator ones column (f32 bitcast: walrus rejects
        # f32r-dtype memset; bit pattern of 1.0f is the same either way)
        nc.vector.memset(vp_sb.bitcast(F32)[:, :, HD:HD1], 1.0)

        # ---- stream context blocks: K^T chunks + V s-tiles ----
        for sb in range(NSB):
            cx = ctxp.tile([P, KC, SBLK], F32R, tag="ctx", name="cx")
            nc.sync.dma_start(
                out=cx,
                in_=ctxT.rearrange("(k p) s -> p k s", p=P)[
                    :, :, sb * SBLK:(sb + 1) * SBLK
                ],
            )
            for chn in range(NKTC):
                kp = pp.tile([P, KTW], F32, tag="proj", name="kp")
                for k in range(KC):
                    nc.tensor.matmul(
                        kp,
                        lhsT=wk_sb[:, k, :],
                        rhs=cx[:, k, chn * KTW:(chn + 1) * KTW],
                        start=(k == 0),
                        stop=(k == KC - 1),
                    )
                off = sb * SBLK + chn * KTW
                nc.vector.tensor_copy(kt_sb[:, off:off + KTW], kp)
            for st in range(STB):
                vps = pp.tile([P, HD], F32, tag="proj", name="vps")
                for k in range(KC):
                    nc.tensor.matmul(
                        vps,
                        lhsT=cx[:, k, st * P:(st + 1) * P],
                        rhs=wv_sb[:, k, :],
                        start=(k == 0),
                        stop=(k == KC - 1),
                    )
                nc.vector.tensor_copy(vp_sb[:, sb * STB + st, 0:HD], vps)

        # ---- fused attention ----
        for tch in range(NTCH):
            avs = [
                pa.tile([P, HD1], F32, tag="av", name=f"av{i}") for i in range(T128)
            ]
            for st in range(ST):
                scp = ps.tile([P, TCW], F32, tag="sc", name="scp")
                nc.tensor.matmul(
                    scp,
                    lhsT=kt_sb[:, st * P:(st + 1) * P],
                    rhs=qt_sb[:, tch * TCW:(tch + 1) * TCW],
                    start=True,
                    stop=True,
                )
                ex = expp.tile([P, TCW], F32R, tag="exp", name="ex")
                nc.scalar.activation(
                    ex, scp, mybir.ActivationFunctionType.Exp, scale=0.125
                )
                for i in range(T128):
                    nc.tensor.matmul(
                        avs[i],
                        lhsT=ex[:, i * P:(i + 1) * P],
                        rhs=vp_sb[:, st, :],
                        start=(st == 0),
                        stop=(st == ST - 1),
                    )
            for i in range(T128):
                rc = outp.tile([P, 1], F32, tag="rc", name="rc")
                nc.vector.reciprocal(rc, avs[i][:, HD:HD + 1])
                ot = outp.tile([P, HD], F32, tag="ot", name="ot")
                nc.vector.tensor_scalar_mul(ot, avs[i][:, 0:HD], rc)
                row = (tch * T128 + i) * P
                nc.sync.dma_start(out=out[row:row + P, :], in_=ot)

    nc.finalize()
    return nc


def make_core_inputs(tokens, context, Wq, Wk, Wv, core):
    """Numpy-side shard prep for one core (layout only, no FLOPs)."""
    b, th = core // 2, core % 2
    KH = (HID + P - 1) // P
    tokTp = np.zeros((KH * P, TC), dtype=np.float32)
    tokTp[:HID] = tokens[b, th * TC:(th + 1) * TC, :].T
    ctxT = np.ascontiguousarray(context[b].T)
    wqT = np.zeros((KH * P, P), dtype=np.float32)
    wqT[:HID, :E] = Wq.T
    wkT = np.zeros((CTX, P), dtype=np.float32)
    wkT[:, :E] = Wk.T
    wvT = np.ascontiguousarray(Wv.T)
    return {"tokT": tokTp, "ctxT": ctxT, "wqT": wqT, "wkT": wkT, "wvT": wvT}


_NC = None


def kernel(tokens, context, Wq, Wk, Wv):
    global _NC
    tokens = np.asarray(tokens, dtype=np.float32)
    context = np.asarray(context, dtype=np.float32)
    Wq = np.asarray(Wq, dtype=np.float32)
    Wk = np.asarray(Wk, dtype=np.float32)
    Wv = np.asarray(Wv, dtype=np.float32)

    if _NC is None:
        _NC = build_cross_attn()

    in_maps = [
        make_core_inputs(tokens, context, Wq, Wk, Wv, c) for c in range(NCORES)
    ]
    res = run_bass_kernel_spmd(_NC, in_maps, core_ids=list(range(NCORES)))

    out = np.empty((B, T, HID), dtype=np.float32)
    for c in range(NCORES):
        b, th = c // 2, c % 2
        out[b, th * TC:(th + 1) * TC, :] = res.results[c]["out"]
    return out
